# revision 22
# baseline (speedup 1.0000x reference)
"""Trainium2 Bass kernel for nn_H_DYNA_42348377538865 (scatter_memory GRU + memory attention).

Self-contained: shards node dim N=512 across 8 NeuronCores (64 nodes/core),
runs a fully-unrolled 24-step recurrence per core, gathers on host.

Layout: feature-on-partitions, (node, batch) on free dim (col = n_local*32 + b,
NB=2048 cols/core). Key decompositions (validated vs reference in numpy):
  - rolling q-cache: q(h_t) computed once/step; 12 slots in 3x[128,2048] tiles;
    slot j pairs with memory slice s=(j-t)%12 -> 12 precomputed rotation stacks
  - softmax without max-subtraction; fused-mean matmul with M=128 emitting
    [lc;gc] rows 0:64 and replicated sums rows 64:128 so reciprocal is
    broadcast-ready without an extra matmul
  - hypernet nsw = node_emb @ weight_pool precomputed on host (param repack)
  - decode is autoregressive: x_{d+1} = y_d
"""
import numpy as np
import sys

for _p in ("/opt/trn_rl_repo",):
    if _p not in sys.path:
        sys.path.append(_p)

import concourse.bass as bass
import concourse.bacc as bacc
import concourse.mybir as mybir
import concourse.tile as tile
from concourse import bass_utils

B, T, HORIZON, N = 32, 12, 12, 512
IN, OUT, H, P = 1, 1, 64, 32
S, ML, MG, DE = 12, 64, 32, 10
NCORES = 8
NL = N // NCORES        # 64
NB = NL * B             # 2048
NSTEP = T + HORIZON     # 24
CH = 4                  # column chunks
CW = NB // CH           # 512

F32 = mybir.dt.float32
F32R = mybir.dt.float32r
BF16 = mybir.dt.bfloat16
AF = mybir.ActivationFunctionType

CTX_BF16 = True        # nsw + fn in bf16: ctx matmuls 1 cyc/row instead of 4
ADD_ON_GPSIMD = True   # h-update add on gpsimd to unload DVE


def build_nc():
    nc = bacc.Bacc("TRN2", target_bir_lowering=False, debug=False)
    d = {}
    d["xsrc"] = nc.dram_tensor("xsrc", [T, NB], BF16, kind="ExternalInput")
    d["memstack"] = nc.dram_tensor("memstack", [128, S * 3 * 96], BF16, kind="ExternalInput")
    d["nsw"] = nc.dram_tensor("nsw", [64, NL * 64], BF16, kind="ExternalInput")
    d["fmean"] = nc.dram_tensor("fmean", [96, 64], BF16, kind="ExternalInput")
    d["fsum"] = nc.dram_tensor("fsum", [96, 64], BF16, kind="ExternalInput")
    d["zw"] = nc.dram_tensor("zw", [65, 64], BF16, kind="ExternalInput")
    d["rw"] = nc.dram_tensor("rw", [65, 64], BF16, kind="ExternalInput")
    d["cw"] = nc.dram_tensor("cw", [65, 64], BF16, kind="ExternalInput")
    d["qw"] = nc.dram_tensor("qw", [64, 32], BF16, kind="ExternalInput")
    d["ow"] = nc.dram_tensor("ow", [64, 1], BF16, kind="ExternalInput")
    d["bq4"] = nc.dram_tensor("bq4", [128, 1], F32, kind="ExternalInput")
    d["bz"] = nc.dram_tensor("bz", [64, 1], F32, kind="ExternalInput")
    d["br"] = nc.dram_tensor("br", [64, 1], F32, kind="ExternalInput")
    d["bc"] = nc.dram_tensor("bc", [64, 1], F32, kind="ExternalInput")
    d["bo"] = nc.dram_tensor("bo", [1, 1], F32, kind="ExternalInput")
    ys_d = nc.dram_tensor("ys", [HORIZON, NB], BF16, kind="ExternalOutput")

    with tile.TileContext(nc) as tc:
        with (
            tc.tile_pool(name="consts", bufs=1) as cp,
            tc.tile_pool(name="sp", bufs=3) as sp,
            tc.tile_pool(name="pp_lg", bufs=1, space="PSUM") as pp_lg,
            tc.tile_pool(name="pp_fu", bufs=1, space="PSUM") as pp_fu,
            tc.tile_pool(name="pp_su", bufs=1, space="PSUM") as pp_su,
            tc.tile_pool(name="pp_acc", bufs=1, space="PSUM") as pp_acc,
            tc.tile_pool(name="pp_z", bufs=1, space="PSUM") as pp_z,
            tc.tile_pool(name="pp_r", bufs=1, space="PSUM") as pp_r,
            tc.tile_pool(name="pp_qp", bufs=1, space="PSUM") as pp_qp,
            tc.tile_pool(name="pp_yp", bufs=1, space="PSUM") as pp_yp,
        ):
            xs = cp.tile([T, NB], BF16)
            nc.sync.dma_start(xs[:], d["xsrc"].ap())
            msk = cp.tile([128, S * 3 * 96], BF16)
            nc.sync.dma_start(msk[:], d["memstack"].ap())
            nsw = cp.tile([64, NL * 64], BF16)
            nc.sync.dma_start(nsw[:], d["nsw"].ap())
            fmean = cp.tile([96, 64], BF16)
            nc.sync.dma_start(fmean[:], d["fmean"].ap())
            fsum = cp.tile([96, 64], BF16)
            nc.sync.dma_start(fsum[:], d["fsum"].ap())
            zw = cp.tile([65, 64], BF16)
            nc.sync.dma_start(zw[:], d["zw"].ap())
            rw = cp.tile([65, 64], BF16)
            nc.sync.dma_start(rw[:], d["rw"].ap())
            cw = cp.tile([65, 64], BF16)
            nc.sync.dma_start(cw[:], d["cw"].ap())
            qw = cp.tile([64, 32], BF16)
            nc.sync.dma_start(qw[:], d["qw"].ap())
            ow = cp.tile([64, 1], BF16)
            nc.sync.dma_start(ow[:], d["ow"].ap())
            bq4 = cp.tile([128, 1], F32)
            nc.sync.dma_start(bq4[:], d["bq4"].ap())
            bz = cp.tile([64, 1], F32)
            nc.sync.dma_start(bz[:], d["bz"].ap())
            br = cp.tile([64, 1], F32)
            nc.sync.dma_start(br[:], d["br"].ap())
            bc = cp.tile([64, 1], F32)
            nc.sync.dma_start(bc[:], d["bc"].ap())
            bo = cp.tile([1, 1], F32)
            nc.sync.dma_start(bo[:], d["bo"].ap())

            qb = []
            for g in range(3):
                q = cp.tile([128, NB], BF16, name=f"qb{g}")
                nc.vector.memset(q[:], 0.0)
                nc.scalar.activation(q[:], q[:], AF.Identity, bias=bq4[:, 0:1])
                qb.append(q)
            hx = cp.tile([65, NB], BF16)
            nc.vector.memset(hx[:], 0.0)
            rhx = cp.tile([65, NB], BF16)
            nc.vector.memset(rhx[:], 0.0)
            ystage = cp.tile([1, NB], BF16)
            nc.sync.dma_start(hx[64:65, :], xs[0:1, :])
            nc.sync.dma_start(rhx[64:65, :], xs[0:1, :])

            for t in range(NSTEP):
                r = t % S
                j = t % S
                g_w, row_w = j // 4, (j % 4) * 32
                for c in range(CH):
                    cs = slice(c * CW, (c + 1) * CW)
                    lg = pp_lg.tile([96, CW], F32, tag="lg")
                    for g in range(3):
                        off = (r * 3 + g) * 96
                        nc.tensor.matmul(
                            lg[:], msk[:, off : off + 96], qb[g][:, cs],
                            start=(g == 0), stop=(g == 2),
                        )
                    ex = sp.tile([96, CW], BF16, tag="ex")
                    nc.scalar.activation(ex[:], lg[:], AF.Exp)
                    fu = pp_fu.tile([64, CW], F32, tag="fu")
                    nc.tensor.matmul(fu[:], fmean[:], ex[:], start=True, stop=True)
                    su = pp_su.tile([64, CW], F32, tag="su")
                    nc.tensor.matmul(su[:], fsum[:], ex[:], start=True, stop=True)
                    rt = sp.tile([64, CW], F32, tag="rt")
                    nc.vector.reciprocal_approx_fast(rt[:], su[:])
                    fn = sp.tile([64, CW], BF16, tag="fn")
                    nc.vector.tensor_mul(fn[:], fu[:], rt[:])
                    zp = pp_z.tile([64, CW], F32, tag="zp")
                    nc.tensor.matmul(zp[:], zw[:], hx[:, cs], start=True, stop=True)
                    rp = pp_r.tile([64, CW], F32, tag="rp")
                    nc.tensor.matmul(rp[:], rw[:], hx[:, cs], start=True, stop=True)
                    zt = sp.tile([64, CW], F32, tag="zt")
                    nc.scalar.activation(zt[:], zp[:], AF.Sigmoid, bias=bz[:, 0:1])
                    rs = sp.tile([64, CW], F32, tag="rs")
                    nc.scalar.activation(rs[:], rp[:], AF.Sigmoid, bias=br[:, 0:1])
                    nc.vector.tensor_mul(rhx[0:64, cs], rs[:], hx[0:64, cs])
                    acc = pp_acc.tile([64, CW], F32, tag="acc")
                    nc.tensor.matmul(
                        acc[:], cw[:], rhx[:, cs],
                        start=True, stop=False, skip_group_check=True,
                    )
                    for k in range(16):
                        n = c * 16 + k
                        nsw_ap = nsw[:, n * 64 : (n + 1) * 64]
                        fn_ap = fn[:, k * 32 : (k + 1) * 32]
                        if not CTX_BF16:
                            nsw_ap, fn_ap = nsw_ap, fn_ap
                        nc.tensor.matmul(
                            acc[:, k * 32 : (k + 1) * 32], nsw_ap, fn_ap,
                            start=False, stop=(k == 15), skip_group_check=True,
                        )
                    hc = sp.tile([64, CW], F32, tag="hc")
                    nc.scalar.activation(hc[:], acc[:], AF.Tanh, bias=bc[:, 0:1])
                    dl = sp.tile([64, CW], F32, tag="dl")
                    nc.gpsimd.tensor_sub(dl[:], hc[:], hx[0:64, cs])
                    nc.vector.tensor_mul(dl[:], zt[:], dl[:])
                    add_eng = nc.gpsimd if ADD_ON_GPSIMD else nc.vector
                    add_eng.tensor_add(hx[0:64, cs], hx[0:64, cs], dl[:])
                    # q(h_t) lands directly on the target qbuf slot partitions
                    qp = pp_qp.tile([128, CW], F32, tag="qp")
                    nc.tensor.matmul(
                        qp[row_w : row_w + 32, :], qw[:], hx[0:64, cs],
                        start=True, stop=True, tile_position=(0, row_w),
                    )
                    nc.scalar.activation(
                        qb[g_w][row_w : row_w + 32, cs], qp[row_w : row_w + 32, :],
                        AF.Identity, bias=bq4[row_w : row_w + 32, 0:1],
                    )
                    if t >= T:
                        yp = pp_yp.tile([1, CW], F32, tag="yp")
                        nc.tensor.matmul(yp[:], ow[:], hx[0:64, cs], start=True, stop=True)
                        nc.scalar.activation(ystage[0:1, cs], yp[0:1, :], AF.Identity, bias=bo[0:1, 0:1])
                if t < T - 1:
                    nc.sync.dma_start(hx[64:65, :], xs[t + 1 : t + 2, :])
                    nc.sync.dma_start(rhx[64:65, :], xs[t + 1 : t + 2, :])
                elif t >= T:
                    dstep = t - T
                    nc.sync.dma_start(ys_d[dstep : dstep + 1, :], ystage[0:1, :])
                    if t < NSTEP - 1:
                        nc.sync.dma_start(hx[64:65, :], ystage[0:1, :])
                        nc.sync.dma_start(rhx[64:65, :], ystage[0:1, :])
    nc.compile()
    return nc


def precompute(inp):
    lm = np.asarray(inp["local_mem"], np.float32)
    gm = np.asarray(inp["global_mem"], np.float32)
    Wq = np.asarray(inp["Wq"], np.float32)
    bq = np.asarray(inp["bq"], np.float32)
    node_emb = np.asarray(inp["node_emb"], np.float32)
    wp = np.asarray(inp["weight_pool"], np.float32)
    Wz = np.asarray(inp["Wz"], np.float32)
    bz = np.asarray(inp["bz"], np.float32)
    Wr = np.asarray(inp["Wr"], np.float32)
    br = np.asarray(inp["br"], np.float32)
    Wc = np.asarray(inp["Wc"], np.float32)
    bc = np.asarray(inp["bc"], np.float32)
    Wo = np.asarray(inp["Wo"], np.float32)
    bo = np.asarray(inp["bo"], np.float32)

    c = {}
    c["nsw_full"] = np.einsum("nd,dfh->nfh", node_emb, wp).astype(np.float32)
    memsl = np.concatenate([lm.transpose(2, 0, 1), gm.transpose(2, 0, 1)], axis=1)  # [P,96,S]
    ms = np.zeros((128, S, 3, 96), np.float32)
    for r in range(S):
        for g in range(3):
            for i in range(4):
                s = (4 * g + i - r) % S
                ms[32 * i : 32 * (i + 1), r, g, :] = memsl[:, :, s]
    c["memstack"] = ms.reshape(128, S * 3 * 96)
    lmean, gmean = lm.mean(axis=1), gm.mean(axis=1)
    fs = np.zeros((96, 64), np.float32)
    fs[:ML, :P] = lmean
    fs[ML:, P : 2 * P] = gmean
    c["fmean"] = fs
    fsum = np.zeros((96, 64), np.float32)
    fsum[:ML, :P] = 1.0
    fsum[ML:, P : 2 * P] = 1.0
    c["fsum"] = fsum
    zwm = np.zeros((H + 1, H), np.float32)
    zwm[:H] = Wz[1:]
    zwm[H] = Wz[0]
    c["zw"] = zwm
    rwm = np.zeros((H + 1, H), np.float32)
    rwm[:H] = Wr[1:]
    rwm[H] = Wr[0]
    c["rw"] = rwm
    cc = np.zeros((H + 1, H), np.float32)
    cc[:H] = Wc[1:]
    cc[H] = Wc[0]
    c["cw"] = cc
    c["qw"] = Wq.copy()
    c["ow"] = Wo[:, 0:1].copy()
    c["bq4"] = np.tile(bq, 4).reshape(128, 1)
    c["bz"] = bz.reshape(64, 1)
    c["br"] = br.reshape(64, 1)
    c["bc"] = bc.reshape(64, 1)
    c["bo"] = bo.reshape(1, 1)
    return c


def _bf16(a):
    import ml_dtypes
    return np.ascontiguousarray(a).astype(ml_dtypes.bfloat16)


def make_in_maps(inp):
    c = precompute(inp)
    src = np.asarray(inp["source"], np.float32)
    shared = {
        "memstack": _bf16(c["memstack"]), "fmean": _bf16(c["fmean"]),
        "fsum": _bf16(c["fsum"]), "zw": _bf16(c["zw"]), "rw": _bf16(c["rw"]),
        "cw": _bf16(c["cw"]), "qw": _bf16(c["qw"]), "ow": _bf16(c["ow"]),
        "bq4": c["bq4"], "bz": c["bz"], "br": c["br"], "bc": c["bc"], "bo": c["bo"],
    }
    in_maps = []
    for core in range(NCORES):
        nodes = slice(core * NL, (core + 1) * NL)
        xs = _bf16(src[:, :, nodes, 0].transpose(1, 2, 0).reshape(T, NB))
        nswc = _bf16(c["nsw_full"][nodes].transpose(1, 0, 2).reshape(64, NL * 64))
        in_maps.append(dict(shared, xsrc=xs, nsw=nswc))
    return in_maps


def assemble(results):
    out = np.zeros((B, HORIZON, N, OUT), np.float32)
    for core in range(NCORES):
        nodes = slice(core * NL, (core + 1) * NL)
        ys = np.asarray(results[core]["ys"], np.float32)  # [HORIZON, NB]
        out[:, :, nodes, 0] = ys.reshape(HORIZON, NL, B).transpose(2, 0, 1)
    return out


_NC_CACHE = {}


def kernel(**inputs):
    if "nc" not in _NC_CACHE:
        _NC_CACHE["nc"] = build_nc()
    nc = _NC_CACHE["nc"]
    in_maps = make_in_maps(inputs)
    res = bass_utils.run_bass_kernel_spmd(nc, in_maps, core_ids=list(range(NCORES)))
    return assemble(res.results)
